# revision 37
# baseline (speedup 1.0000x reference)
"""AdaptiveTemporalKernels Trainium2 kernel (v2).

Strategy: data-parallel over batch (B=8 -> 1 element/core, zero collectives).
Weights host-side pre-transposed / pre-tiled / DoubleRow-pair-packed / cast
to fp8e4 (x128 scale); activations catT/aoT/ao2T in fp8 (x128) with
256-column branch stride so DoubleRow pair APs are 16B-aligned. Big matmuls
(qkv, attn_out, proj) run fp8 DoubleRow (K=256/instr) with f32 PSUM.

v2 changes vs baseline:
 - Depthwise convs run in bf16 on VectorE (2x_1P mode) with host-prepadded
   xpad0/xpad1 (1-col-shifted twin) so every tap slice is 4B-aligned;
   k3/k5 branches run on GpSimd in parallel; catT scale-writes moved to
   ScalarE (activation Copy with per-partition aw scale).
 - Q projection chase dropped: all-DoubleRow (PE work -37us).
 - All K=1 bias-broadcast matmuls removed: q/k/ao2 biases ride the
   PSUM->SBUF cast (DVE tensor_scalar const-mult + AP-add, or ScalarE
   Identity with AP bias); V bias folded host-side into the attn-out bias
   (softmax rows sum to 1); proj bias keeps its 2 tiny ones-matmuls.
 - Attention softmax batched: scores+exp for all heads, then all colsums,
   then per-head broadcast+aoT -- removes 8x ~1.9us PE stalls and the HAM
   re-throttle they caused.
 - proj runs token-block 0 fully before block 1 so layernorm 0 overlaps
   block 1's matmuls.
"""
import os
import sys

sys.path.insert(0, "/opt/trn_rl_repo")

import numpy as np
import ml_dtypes

import concourse.bass as bass
import concourse.tile as tile
from concourse import mybir
from concourse.bass_utils import run_bass_kernel_spmd

BF16 = mybir.dt.bfloat16
F32 = mybir.dt.float32
FP8 = mybir.dt.float8e4
DR = mybir.MatmulPerfMode.DoubleRow
AFT = mybir.ActivationFunctionType
ALU = mybir.AluOpType

KS = [3, 5, 7, 9, 11]
ND = 8
D = 128
E = 5120
H = 8
B = 8
S = 250
HD = E // H          # 640
NE = E // 128        # 40 e-tiles
CST = 256            # catT/aoT/ao2T per-branch column stride (16B-aligned pairs)
SCALE = 1.0 / float(np.sqrt(HD))
PAD = 40             # max conv halo: (11-1)*8//2
XPW = 332            # xpad width (S + 2*PAD rounded up even)
N_CORES = 8
WS = 128.0           # fp8 weight scale
AS = 128.0           # fp8 activation scale
UNS = 1.0 / (WS * AS)  # 2^-14 psum unscale

# Branch order along the catT e-axis. The k=11 and k=9 branches run on the
# TensorEngine (accumulated diagonal-matmul taps, pairs 0-7) so the Q
# projection's first pass has ready pairs immediately; the remaining
# branches run on VectorE (bf16 taps at ~425ns each), paired (k3,k7) and
# (k5,k5) so every DVE pair costs the same (8 accumulate-taps).
_PERM = []
for _d in range(ND):
    _PERM += [4 * ND + _d, 3 * ND + _d]   # k=11 with k=9 (PE), pairs 0-7
for _d in range(ND):
    _PERM += [0 * ND + _d, 2 * ND + _d]   # k=3 with k=7 (DVE)
for _d in range(0, ND, 2):
    _PERM += [1 * ND + _d, 1 * ND + _d + 1]  # k=5 with k=5 (DVE)
assert sorted(_PERM) == list(range(40))
NPE = 16  # leading k11/k9 branches produced on the TensorEngine
# flat tap-matrix index base per PE branch (k alternates 11, 9)
_WDOFF = [0]
for _p in range(1, NPE + 1):
    _WDOFF.append(_WDOFF[-1] + (11 if _p % 2 == 1 else 9))
NWD = _WDOFF[NPE]  # 160

# All small constants ride ONE packed [128, CBLK_W] f32 DMA — ~18 separate
# small transfers each pay ~1.5us of ring/completion latency otherwise.
CBLK_LAYOUT = [("cw3", 24), ("cw5", 40), ("cw7", 56), ("cw9", 72), ("cw11", 88),
               ("cb3", 8), ("cb5", 8), ("cb7", 8), ("cb9", 8), ("cb11", 8),
               ("kg1", 128), ("kgb1", 1), ("kgb2", 40),
               ("bqc", 40), ("bkc", 40), ("boc", 40),
               ("gam", 128), ("bet", 128)]
CBLK_OFF = {}
_acc = 0
for _n, _w in CBLK_LAYOUT:
    CBLK_OFF[_n] = _acc
    _acc += _w
CBLK_W = _acc

LAST_RESULT = None
_NC_CACHE = None


def _split_multi_waits(nc, max_waits=1):
    """This container's walrus only lowers ONE sync-wait per instruction.
    Split any instruction carrying N>1 waits into N-1 preceding single-wait
    NoOps on the same engine."""
    import bass_rust
    SyncInfo = bass_rust.SyncInfo
    n_split = 0
    for f in nc.m.functions:
        for bb in f.blocks:
            insts = bb.instructions
            i = 0
            while i < len(insts):
                inst = insts[i]
                si = getattr(inst, "sync_info", None)
                if si is not None and si.on_wait is not None and len(si.on_wait) > max_waits:
                    waits = list(si.on_wait)
                    keep, extra = waits[-max_waits:], waits[:-max_waits]
                    nops = []
                    for w in extra:
                        nop = mybir.InstNoOp(name=f"WSPLIT-{nc.next_id()}", ins=[], outs=[])
                        nop.engine = inst.engine
                        nop.sync_info = SyncInfo(on_wait=[w], on_update=[])
                        nops.append(nop)
                    inst.sync_info = SyncInfo(on_wait=keep, on_update=list(si.on_update))
                    insts[i:i] = nops
                    i += len(nops)
                    n_split += 1
                i += 1
    return n_split


def _maybe_install_trace_shim():
    """Register the NTFF profile hook (missing antenv.axon_hooks in this image)
    so BASS_TRACE=1 yields exec_time_ns. Only used by test.py runs."""
    if not os.environ.get("BASS_TRACE"):
        return
    import types
    import antenv
    if "antenv.axon_hooks" not in sys.modules:
        mod = types.ModuleType("antenv.axon_hooks")
        mod._hook = None
        def set_axon_ntff_profile_hook(h):
            mod._hook = h
        def get_axon_ntff_profile_hook():
            return mod._hook
        mod.set_axon_ntff_profile_hook = set_axon_ntff_profile_hook
        mod.get_axon_ntff_profile_hook = get_axon_ntff_profile_hook
        sys.modules["antenv.axon_hooks"] = mod
        antenv.axon_hooks = mod
    from antenv.axon_hooks import set_axon_ntff_profile_hook
    from trn_agent_boot.trn_boot import _ntff_profile_via_ctypes
    set_axon_ntff_profile_hook(_ntff_profile_via_ctypes("/opt/axon/libaxon_pjrt.so"))
    from concourse import bass_utils
    bass_utils.upload_artifacts = lambda tmpdir: f"file://{tmpdir}"


def build_nc():
    nc = bass.Bass()

    x_ext = nc.declare_dram_parameter("x", [S, D], F32, False)
    xp0_ext = nc.declare_dram_parameter("xp0", [D, XPW], BF16, False)
    wq_ext = nc.declare_dram_parameter("wq", [10, 10, 128, 2048], FP8, False)
    wk_ext = nc.declare_dram_parameter("wk", [10, 10, 128, 2048], FP8, False)
    wv_ext = nc.declare_dram_parameter("wv", [10, 10, 128, 2048], FP8, False)
    wo_ext = nc.declare_dram_parameter("wo", [10, 10, 128, 2048], FP8, False)
    wp_ext = nc.declare_dram_parameter("wp", [128, 5120], FP8, False)
    kg2_ext = nc.declare_dram_parameter("kg2", [128, E], BF16, False)
    wd_ext = nc.declare_dram_parameter("wdiag", [128, NWD * 128], BF16, False)
    cblk_ext = nc.declare_dram_parameter("cblk", [128, CBLK_W], F32, False)
    bp_ext = nc.declare_dram_parameter("bp", [1, 128], BF16, False)
    out_ext = nc.declare_dram_parameter("out", [S, D], F32, True)

    TBLK = [(0, 128, 0), (1, 122, 128)]  # (idx, tok_len, tok_offset)

    with tile.TileContext(nc) as tc:
        with (
            tc.tile_pool(name="const", bufs=1) as cpool,
            tc.tile_pool(name="big", bufs=1) as bpool,
            tc.tile_pool(name="w", bufs=16) as wpool,
            tc.tile_pool(name="y", bufs=8) as ypool,
            tc.tile_pool(name="exp", bufs=8) as epool,
            tc.tile_pool(name="rec", bufs=8) as rpool,
            tc.tile_pool(name="ln", bufs=2) as lpool,
            tc.tile_pool(name="ps", bufs=8, space="PSUM") as pspool,
        ):
            mm = nc.tensor.matmul

            def pairv(ap_base, f, c0, c1):
                """[128, 2, c1-c0] DoubleRow view of adjacent CST-strided
                feature blocks (f, f+1)."""
                v = ap_base[:, f * CST:(f + 2) * CST]
                v = v.rearrange("p (j n) -> p j n", j=2)
                return v[:, :, c0:c1]

            # ---- constants / small inputs -------------------------------
            ones_f = cpool.tile([128, 128], F32, tag="ones_f")
            nc.vector.memset(ones_f[:], 1.0)
            scr = cpool.tile([1, 4], F32, tag="scr")
            # pre-warm the gelu act-table set while input DMAs land
            nc.scalar.activation(scr[0:1, 0:1], ones_f[0:1, 0:1], AFT.Gelu)
            ones_h = cpool.tile([128, 128], BF16, tag="ones_h")
            nc.vector.memset(ones_h[:], 1.0)
            ones_8 = cpool.tile([128, 1], FP8, tag="ones_8")
            nc.vector.memset(ones_8[:], 1.0)
            eps_sb = cpool.tile([128, 1], F32, tag="eps")
            nc.vector.memset(eps_sb[:], 1e-5)

            # DMA order is tuned to the consumption schedule: x/xp/cblk
            # first (convs start ~2us in), wdiag on the scalar ring in
            # parallel, kg2 next (the aw chain); wp is deferred until after
            # the attention section is emitted.
            x_tok = cpool.tile([128, 256], F32, tag="x_tok")
            nc.sync.dma_start(x_tok[0:128, 0:128], x_ext[0:128, :])
            nc.sync.dma_start(x_tok[0:122, 128:256], x_ext[128:250, :])
            xp0 = cpool.tile([128, XPW], BF16, tag="xp0")
            nc.sync.dma_start(xp0[:], xp0_ext[:])
            cblk = cpool.tile([128, CBLK_W], F32, tag="cblk")
            nc.sync.dma_start(cblk[:], cblk_ext[:])

            def cbk(name):
                return cblk[:, CBLK_OFF[name]:CBLK_OFF[name] + dict(CBLK_LAYOUT)[name]]

            cw_sb = {k: cbk(f"cw{k}") for k in KS}
            cb_sb = {k: cbk(f"cb{k}") for k in KS}
            kg1_sb = cbk("kg1")
            kgb1_sb = cbk("kgb1")
            kgb2_sb = cbk("kgb2")
            bqc_sb = cbk("bqc")
            bkc_sb = cbk("bkc")
            boc_sb = cbk("boc")
            gam_sb = cbk("gam")
            bet_sb = cbk("bet")
            wd_sb = cpool.tile([128, NWD * 128], BF16, tag="wd")
            for gp in range(0, ND, 2):
                o0, o1 = _WDOFF[2 * gp] * 128, _WDOFF[2 * gp + 4] * 128
                nc.scalar.dma_start(wd_sb[:, o0:o1], wd_ext[:, o0:o1])
            kg2_sb = cpool.tile([128, E], BF16, tag="kg2")
            nc.sync.dma_start(kg2_sb[:], kg2_ext[:])
            bp_sb = cpool.tile([1, 128], BF16, tag="bp")
            nc.sync.dma_start(bp_sb[:], bp_ext[:])

            # ---- big persistent SBUF tensors ----------------------------
            catT = bpool.tile([128, NE * CST], FP8, tag="catT")    # [e, tok] x128
            qT = bpool.tile([128, NE * CST], FP8, tag="qT")    # (q+bq)*SCALE*AS
            kT = bpool.tile([128, NE * CST], FP8, tag="kT")    # (k+bk)*AS
            v_sb = bpool.tile([128, 2 * E], FP8, tag="v")      # [tok, blk*E+f] x128
            aoT = bpool.tile([128, NE * CST], FP8, tag="aoT")      # x128
            wp_all = bpool.tile([128, 5120], FP8, tag="wp_all")
            ao2T = bpool.tile([128, NE * CST], FP8, tag="ao2T")    # x128

            # ---- kernel generator (pt 1): h = gelu(W1 mean(x)) ----------
            ps_g = pspool.tile([128, 1], F32, tag="mm")
            mm(ps_g[:], x_tok[0:128, 0:128], ones_f[0:128, 0:1], start=True, stop=False)
            mm(ps_g[:], x_tok[0:122, 128:256], ones_f[0:122, 0:1], start=False, stop=True)
            gT = lpool.tile([128, 1], F32, tag="gT")
            nc.scalar.activation(gT[:], ps_g[:], AFT.Copy, scale=1.0 / S)
            ps_h = pspool.tile([128, 1], F32, tag="mm")
            mm(ps_h[:], kg1_sb[:], gT[:], start=True, stop=True)
            hT = lpool.tile([128, 1], BF16, tag="hT")
            nc.scalar.activation(hT[:], ps_h[:], AFT.Gelu, bias=kgb1_sb[:, 0:1])
            awT = cpool.tile([128, NE], F32, tag="awT")
            awpre = lpool.tile([128, NE], F32, tag="awpre")
            cbaw = lpool.tile([128, NPE], F32, tag="cbaw")

            # ---- depthwise convs -> catT (feature-major, fp8 x128) ------
            # k11/k9 branches (catT pairs 0-7) run on the TensorEngine as
            # accumulated diagonal matmuls; the rest run on VectorE in bf16,
            # the init tap alternating DVE/ScalarE and the catT scale-write
            # on ScalarE. Emission order is hand-scheduled so no engine FIFO
            # ever waits on the aw chain.
            ps_y = {}

            def conv_pe(p):
                k = 11 if p % 2 == 0 else 9
                dil = p // 2 + 1
                b0 = PAD - (k - 1) * dil // 2
                t = pspool.tile([128, S], F32, tag="mm", name=f"ps_y{p}")
                for j in range(k):
                    a = _WDOFF[p] + j
                    mm(t[:], wd_sb[:, a * 128:(a + 1) * 128],
                       xp0[:, b0 + j * dil:b0 + j * dil + S],
                       start=(j == 0), stop=(j == k - 1))
                ps_y[p] = t

            def conv_pe_write(p):
                nc.scalar.activation(catT[:, p * CST:p * CST + S], ps_y[p][:],
                                     AFT.Identity, scale=awT[:, p:p + 1],
                                     bias=cbaw[:, p:p + 1])

            def conv_dve(pos):
                ki, di = _PERM[pos] // ND, _PERM[pos] % ND
                k = KS[ki]
                dil = di + 1
                b0 = PAD - (k - 1) * dil // 2
                y = ypool.tile([128, S], BF16, tag="y")
                if pos % 2 == 0:
                    nc.scalar.activation(y[:], xp0[:, b0:b0 + S], AFT.Identity,
                                         scale=cw_sb[k][:, di * k:di * k + 1],
                                         bias=cb_sb[k][:, di:di + 1])
                else:
                    nc.vector.tensor_scalar(y[:], xp0[:, b0:b0 + S],
                                            cw_sb[k][:, di * k:di * k + 1],
                                            cb_sb[k][:, di:di + 1],
                                            ALU.mult, ALU.add)
                for j in range(1, k):
                    bj = b0 + j * dil
                    nc.vector.scalar_tensor_tensor(y[:], xp0[:, bj:bj + S],
                                                   cw_sb[k][:, di * k + j:di * k + j + 1],
                                                   y[:], ALU.mult, ALU.add)
                return y

            def conv_dve_write(pos, y):
                nc.scalar.activation(catT[:, pos * CST:pos * CST + S], y[:],
                                     AFT.Copy, scale=awT[:, pos:pos + 1])

            conv_pe(0)
            conv_pe(1)
            dpend = [(pos, conv_dve(pos)) for pos in range(NPE, NPE + 4)]
            # kernel generator (pt 2): aw = tanh(W2 h + b2) — the 40 matmuls
            # land between PE conv branches so the PE never idles on gelu
            ps_aw = pspool.tile([128, NE], F32, tag="mm")
            for blk in range(NE):
                mm(ps_aw[:, blk:blk + 1], kg2_sb[:, blk * 128:(blk + 1) * 128], hT[:],
                   start=True, stop=True)
            nc.vector.tensor_add(awpre[:], ps_aw[:], kgb2_sb[:])
            nc.scalar.activation(awT[:], awpre[:], AFT.Tanh)
            # cbaw[:, p] = awT[:, p] * cb[branch p] for the PE branches
            cbaw_v = cbaw.rearrange("p (a b) -> p a b", b=2)
            awT_v = awT[:, 0:NPE].rearrange("p (a b) -> p a b", b=2)
            nc.vector.tensor_mul(cbaw_v[:, :, 0], awT_v[:, :, 0], cb_sb[11][:, 0:ND])
            nc.vector.tensor_mul(cbaw_v[:, :, 1], awT_v[:, :, 1], cb_sb[9][:, 0:ND])
            conv_pe(2)
            conv_pe(3)
            for p in range(4):
                conv_pe_write(p)
            for p in range(4, NPE):
                conv_pe(p)
                conv_pe_write(p)
            for pos, y in dpend:
                conv_dve_write(pos, y)
            # short HAM warmup bridge across the catT-write drain
            ps_w = pspool.tile([128, 128], F32, tag="mm", name="warm")
            for i in range(4):
                mm(ps_w[:], ones_f[:], ones_f[:], start=True, stop=True)
            for pos in range(NPE + 4, NE):
                y = conv_dve(pos)
                conv_dve_write(pos, y)
            # pre-warm the exp act-table set (used next in attention) while
            # the projections run; Copy/Identity are fillers in every set
            nc.scalar.activation(scr[0:1, 1:2], ones_f[0:1, 0:1], AFT.Exp)

            # ---- qT / kT / ao2T: feature-major fp8 DoubleRow ------------
            def qk_like(wext, bias_col, dest, cast_s, src_act, NG=4):
                # NG g-groups interleaved in the PE stream so the conv-paced
                # prologue always has ready matmul work; 2 fblks share one
                # PSUM bank at 256-column halves ([128,512] f32 = 1 bank);
                # bias + unscale ride the PSUM->SBUF cast, alternating
                # DVE (tensor_scalar mult-const + add-AP) and ScalarE
                # (Identity with AP bias) per feature block.
                for g0 in range(0, 10, NG):
                    gs = list(range(g0, min(10, g0 + NG)))
                    ps = {g: [pspool.tile([128, 512], F32, tag="mm",
                                          name=f"ps_{dest.tensor.name}_{g}_{i}")
                              for i in range(2)] for g in gs}
                    for e4 in range(10):
                        wts = {}
                        for gi, g in enumerate(gs):
                            wt = wpool.tile([128, 2048], FP8, tag="w")
                            eng = nc.sync if gi % 2 == 0 else nc.scalar
                            eng.dma_start(wt[:], wext[g, e4])
                            wts[g] = wt
                        for ep in range(2):
                            pr = e4 * 2 + ep
                            rhs = pairv(src_act, 2 * pr, 0, S)
                            for g in gs:
                                for j in range(4):
                                    lh = wts[g][:, ep * 1024 + j * 256:ep * 1024 + (j + 1) * 256]
                                    lh = lh.rearrange("p (j n) -> p j n", j=2)
                                    mm(ps[g][j // 2][:, (j % 2) * 256:(j % 2) * 256 + S],
                                       lh, rhs,
                                       start=(pr == 0), stop=(pr == 19), perf_mode=DR,
                                       skip_group_check=True)
                    for g in gs:
                        for i in range(2):
                            for h2 in range(2):
                                fb = g * 4 + i * 2 + h2
                                sl_d = dest[:, fb * CST:fb * CST + S]
                                sl_p = ps[g][i][:, h2 * 256:h2 * 256 + S]
                                if fb % 2 == 0:
                                    nc.vector.tensor_scalar(
                                        sl_d, sl_p, cast_s, bias_col[:, fb:fb + 1],
                                        ALU.mult, ALU.add)
                                else:
                                    nc.scalar.activation(
                                        sl_d, sl_p, AFT.Identity,
                                        bias=bias_col[:, fb:fb + 1], scale=cast_s)

            qk_like(wq_ext, bqc_sb, qT, UNS * SCALE * AS, catT)
            qk_like(wk_ext, bkc_sb, kT, UNS * AS, catT)

            # ---- V: token-major fp8 DoubleRow (lhsT = catT pairs) -------
            # v stored fp8 at x128 scale (psum * 2^-7); bias folded into the
            # attn-out bias host-side (softmax rows sum to 1).
            # dead rows of the short token block must be finite zeros: the
            # attention DR pair contraction multiplies them by exp's zeroed
            # pad rows, and 0 * garbage-inf would be NaN
            nc.gpsimd.memset(v_sb[96:128, E:2 * E], 0.0)
            for g in range(10):
                psv = [pspool.tile([128, 512], F32, tag="mm", name=f"psv_{g}_{i}") for i in range(2)]
                for e4 in range(10):
                    wt = wpool.tile([128, 2048], FP8, tag="w")
                    eng = nc.sync if e4 % 2 == 0 else nc.scalar
                    eng.dma_start(wt[:], wv_ext[g, e4])
                    for ep in range(2):
                        pr = e4 * 2 + ep
                        rh = wt[:, ep * 1024:(ep + 1) * 1024]
                        rh = rh.rearrange("p (j n) -> p j n", j=2)
                        mm(psv[0][:], pairv(catT, 2 * pr, 0, 128), rh,
                           start=(pr == 0), stop=(pr == 19), perf_mode=DR)
                        mm(psv[1][0:122, :], pairv(catT, 2 * pr, 128, 250), rh,
                           start=(pr == 0), stop=(pr == 19), perf_mode=DR)
                nc.vector.tensor_scalar_mul(
                    v_sb[0:128, g * 512:(g + 1) * 512], psv[0][:], UNS * AS)
                nc.scalar.activation(
                    v_sb[0:122, E + g * 512:E + (g + 1) * 512], psv[1][0:122, :],
                    AFT.Copy, scale=UNS * AS)
            v_pair = v_sb.rearrange("p (j f) -> p j f", j=2)

            # ---- attention (bf16-ish; batched two-pass softmax) ---------
            # Pass A: scoresT + exp for every head; Pass B: all colsums,
            # then per-head reciprocal-broadcast + aoT.
            # Lag-pipelined attention: per iteration i the PE runs scores(i),
            # colsum(i-2), reciprocal-broadcast(i-3) and aoT(i-4), so the
            # softmax chain of each head hides behind other heads' matmuls
            # and the PE busy-density stays high (no HAM re-throttle).
            exs, exns, recips = [], [], []
            for h in range(H):
                ex = epool.tile([128, 512], FP8, tag="exp", name=f"ex{h}")
                # zero the short token-block's dead rows so the DR pair
                # contraction reads 0 * garbage there
                nc.gpsimd.memset(ex[96:128, 256:512], 0.0)
                exs.append(ex)
                exn = epool.tile([128, 512], FP8, tag="exn", name=f"exn{h}")
                nc.gpsimd.memset(exn[96:128, 256:512], 0.0)
                exns.append(exn)

            def att_scores(h):
                for kb, klen, koff in TBLK:
                    ps_s = pspool.tile([128, S], F32, tag="mm", name=f"ps_s{h}_{kb}")
                    for dp in range(2):
                        f = h * 5 + dp * 2
                        mm(ps_s[0:klen, :],
                           pairv(kT, f, koff, koff + klen),
                           pairv(qT, f, 0, S),
                           start=(dp == 0), stop=False, perf_mode=DR,
                           skip_group_check=True)
                    f4 = (h * 5 + 4) * CST
                    mm(ps_s[0:klen, :],
                       kT[:, f4 + koff:f4 + koff + klen],
                       qT[:, f4:f4 + S],
                       start=False, stop=True, skip_group_check=True)
                    nc.scalar.activation(exs[h][0:klen, kb * 256:kb * 256 + S],
                                         ps_s[0:klen, :], AFT.Exp, scale=UNS)

            def att_colsum(h):
                ps_sum = pspool.tile([1, S], F32, tag="mm", name=f"ps_sum{h}")
                mm(ps_sum[:], ones_8[0:128, 0:1], exs[h][0:128, 0:S],
                   start=True, stop=False, skip_group_check=True)
                mm(ps_sum[:], ones_8[0:122, 0:1], exs[h][0:122, 256:256 + S],
                   start=False, stop=True, skip_group_check=True)
                recip = rpool.tile([1, S], F32, tag="recip", name=f"recip{h}")
                nc.vector.reciprocal(recip[:], ps_sum[:])
                recips.append(recip)

            def att_bcast_norm(h):
                ps_b = pspool.tile([128, S], F32, tag="mm", name=f"ps_b{h}")
                mm(ps_b[:], ones_f[0:1, 0:128], recips[h][0:1, :], start=True, stop=True)
                nc.vector.tensor_mul(exns[h][0:128, 0:S], exs[h][0:128, 0:S],
                                     ps_b[0:128, :])
                nc.vector.tensor_mul(exns[h][0:122, 256:256 + S],
                                     exs[h][0:122, 256:256 + S], ps_b[0:122, :])

            def att_ao(h):
                # two dblks share one [128,512] PSUM bank at 256-col halves
                # so each cast covers both; casts alternate ScalarE/DVE so
                # neither engine paces the pipeline below PE speed
                ex_pair = exns[h].rearrange("p (j n) -> p j n", j=2)
                for dd in range(3):
                    nd = 2 if dd < 2 else 1
                    ps_ao = pspool.tile([128, 512], F32, tag="mm", name=f"ps_ao{h}_{dd}")
                    for q in range(nd):
                        dblk = dd * 2 + q
                        c0 = h * HD + dblk * 128
                        mm(ps_ao[:, q * 256:q * 256 + S],
                           v_pair[:, :, c0:c0 + 128], ex_pair[:, :, 0:S],
                           start=True, stop=True, perf_mode=DR, skip_group_check=True)
                    e = h * 5 + dd * 2
                    dst = aoT[:, e * CST:(e + nd) * CST].rearrange("p (j n) -> p j n", j=nd)
                    srcv = ps_ao[:].rearrange("p (j n) -> p j n", j=2)[:, 0:nd, 0:S]
                    if (dd + h) % 2 == 0:
                        nc.scalar.activation(dst[:, :, 0:S], srcv, AFT.Copy, scale=1.0)
                    else:
                        nc.vector.tensor_scalar_mul(dst[:, :, 0:S], srcv, 1.0)

            ps_af = pspool.tile([128, 128], F32, tag="mm", name="attn_fill")
            for i in range(H + 4):
                if i < H:
                    att_scores(i)
                if 0 <= i - 2 < H:
                    att_colsum(i - 2)
                if 0 <= i - 3 < H:
                    att_bcast_norm(i - 3)
                if 0 <= i - 4 < H:
                    att_ao(i - 4)
                mm(ps_af[:], ones_f[:], ones_f[:], start=True, stop=True)
                if i == H - 1:
                    # pre-warm the sqrt act-table set (layernorm) off the
                    # critical tail; Copy/Identity/Square are fillers there
                    nc.scalar.activation(scr[0:1, 2:3], ones_f[0:1, 0:1], AFT.Sqrt)

            # deferred wp DMA (needed only by proj) — lands during the
            # low-DMA attention phase, ahead of ao2's stream
            nc.scalar.dma_start(wp_all[:], wp_ext[:])

            # ---- ao2T: feature-major fp8 DoubleRow (lhsT = Wo pairs) ----
            qk_like(wo_ext, boc_sb, ao2T, UNS * AS, aoT)

            # ---- final proj fp8 DoubleRow + residual + layernorm --------
            # token-block 0's full accumulation chain runs first so its
            # layernorm overlaps block 1's matmuls.
            psf = [pspool.tile([128, 128], F32, tag="mm", name=f"psf_{i}") for i in range(2)]
            ao2_pair = ao2T  # alias for clarity
            for tb, tlen, toff in TBLK:
                for pr in range(20):
                    rh = wp_all[:, pr * 256:(pr + 1) * 256]
                    rh = rh.rearrange("p (j n) -> p j n", j=2)
                    mm(psf[tb][0:tlen, :], pairv(ao2_pair, 2 * pr, toff, toff + tlen), rh,
                       start=(pr == 0), stop=False, perf_mode=DR)
                mm(psf[tb][0:tlen, :], ones_h[0:1, 0:tlen], bp_sb[0:1, :],
                   start=False, stop=True)

                ln_in = lpool.tile([128, 128], F32, tag="ln_in")
                redsum = lpool.tile([128, 1], F32, tag="redsum")
                nc.vector.scalar_tensor_tensor(
                    ln_in[0:tlen, :], psf[tb][0:tlen, :], UNS / AS,
                    x_tok[0:tlen, toff:toff + 128], ALU.mult, ALU.add,
                    accum_out=redsum[0:tlen, :])
                negmean = lpool.tile([128, 1], F32, tag="negmean")
                nc.scalar.activation(negmean[0:tlen, :], redsum[0:tlen, :],
                                     AFT.Copy, scale=-1.0 / D)
                cent = lpool.tile([128, 128], F32, tag="cent")
                nc.vector.tensor_scalar_add(cent[0:tlen, :], ln_in[0:tlen, :],
                                            negmean[0:tlen, 0:1])
                sq = lpool.tile([128, 128], F32, tag="sq")
                varsum = lpool.tile([128, 1], F32, tag="varsum")
                nc.scalar.activation(sq[0:tlen, :], cent[0:tlen, :], AFT.Square,
                                     accum_out=varsum[0:tlen, :])
                std = lpool.tile([128, 1], F32, tag="std")
                nc.scalar.activation(std[0:tlen, :], varsum[0:tlen, :], AFT.Sqrt,
                                     scale=1.0 / D, bias=eps_sb[0:tlen, 0:1])
                rstd = lpool.tile([128, 1], F32, tag="rstd")
                nc.vector.reciprocal(rstd[0:tlen, :], std[0:tlen, :])
                gmm = lpool.tile([128, 128], F32, tag="gmm")
                nc.vector.scalar_tensor_tensor(
                    gmm[0:tlen, :], cent[0:tlen, :], rstd[0:tlen, 0:1],
                    gam_sb[0:tlen, :], ALU.mult, ALU.mult)
                outf = lpool.tile([128, 128], F32, tag="outf")
                nc.vector.tensor_add(outf[0:tlen, :], gmm[0:tlen, :], bet_sb[0:tlen, :])
                nc.sync.dma_start(out_ext[toff:toff + tlen, :], outf[0:tlen, :])

    _split_multi_waits(nc)
    return nc


def _prep_inputs(inputs):
    f32 = lambda a: np.ascontiguousarray(np.asarray(a, dtype=np.float32))
    bf16 = lambda a: np.ascontiguousarray(np.asarray(a, dtype=np.float32).astype(ml_dtypes.bfloat16))
    fp8 = lambda a: np.ascontiguousarray(np.asarray(a, dtype=np.float32).astype(ml_dtypes.float8_e4m3))

    def perm_k(wT):   # permute contraction-axis 128-blocks by _PERM
        n = wT.shape[1]
        return wT.reshape(NE, 128, n)[_PERM].reshape(E, n)

    def dr_lhs(wT, permute=False):  # [E(K), N] -> [g, e4, 128p, (ep,fj,j,m)] DR lhsT pairs
        if permute:
            wT = perm_k(wT)
        n = wT.shape[1]
        return (wT.reshape(10, 2, 2, 128, n // 512, 4, 128)
                .transpose(4, 0, 3, 1, 5, 2, 6).reshape(n // 512, 10, 128, 2048))

    def dr_rhs(wT, permute=False):  # [E(K), N] -> [g, e4, 128p, (ep,j,c)] DR rhs pairs
        if permute:
            wT = perm_k(wT)
        n = wT.shape[1]
        return (wT.reshape(10, 2, 2, 128, n // 512, 512)
                .transpose(4, 0, 3, 1, 2, 5).reshape(n // 512, 10, 128, 2048))

    def col128(vec, scale):  # [E] bias -> [128, NE] per-partition columns
        return f32(np.asarray(vec, np.float32).reshape(NE, 128).T * scale)

    A = np.asarray(inputs["attn_in_w"], dtype=np.float32)
    Wo = np.asarray(inputs["attn_out_w"], np.float32)
    bv = np.asarray(inputs["attn_in_b"], np.float32)[2 * E:3 * E]
    bo_eff = np.asarray(inputs["attn_out_b"], np.float32) + Wo @ bv
    shared = {
        "wq": fp8(dr_lhs(A[0:E].T, permute=True) * WS),
        "wk": fp8(dr_lhs(A[E:2 * E].T, permute=True) * WS),
        "wv": fp8(dr_rhs(A[2 * E:3 * E].T, permute=True) * WS),
        "wo": fp8(dr_lhs(Wo.T) * WS),
        "wp": fp8(np.asarray(inputs["proj_w"], np.float32).T
                  .reshape(10, 2, 2, 128, 128).transpose(3, 0, 1, 2, 4)
                  .reshape(128, 5120) * WS),
        "kg2": bf16(np.asarray(inputs["kg_w2"], np.float32).T.reshape(128, NE, 128)[:, _PERM, :].reshape(128, E)),
        "bp": bf16(np.asarray(inputs["proj_b"]).reshape(1, 128) * WS * AS * AS),
    }
    # packed small-constant block (single DMA); conv taps/bias pre-scaled by
    # AS so y is produced at x128 scale and the catT write is a pure
    # aw-scale copy
    cblk = np.zeros((128, CBLK_W), np.float32)

    def put(name, arr):
        o = CBLK_OFF[name]
        cblk[:, o:o + arr.shape[1]] = arr

    for k in KS:
        put(f"cw{k}", np.asarray(inputs[f"conv_w_k{k}"], np.float32)
            .transpose(1, 0, 2).reshape(128, ND * k) * AS)
        put(f"cb{k}", np.asarray(inputs[f"conv_b_k{k}"], np.float32).T * AS)
    put("kg1", np.asarray(inputs["kg_w1"], np.float32).T)
    put("kgb1", np.asarray(inputs["kg_b1"], np.float32).reshape(128, 1))
    put("kgb2", np.asarray(inputs["kg_b2"], np.float32).reshape(NE, 128)[_PERM].T)
    put("bqc", col128(np.asarray(inputs["attn_in_b"], np.float32)[0:E], SCALE * AS))
    put("bkc", col128(np.asarray(inputs["attn_in_b"], np.float32)[E:2 * E], AS))
    put("boc", col128(bo_eff, AS))
    put("gam", np.broadcast_to(np.asarray(inputs["gamma"], np.float32), (128, 128)))
    put("bet", np.broadcast_to(np.asarray(inputs["beta"], np.float32), (128, 128)))
    shared["cblk"] = f32(cblk)
    # diagonal tap matrices for the PE conv branches (k11/k9 interleaved)
    w11 = np.asarray(inputs["conv_w_k11"], np.float32)  # [ND, 128, 11]
    w9 = np.asarray(inputs["conv_w_k9"], np.float32)    # [ND, 128, 9]
    wd = np.zeros((NWD, 128, 128), np.float32)
    for d in range(ND):
        for j in range(11):
            np.fill_diagonal(wd[_WDOFF[2 * d] + j], w11[d, :, j] * AS)
        for j in range(9):
            np.fill_diagonal(wd[_WDOFF[2 * d + 1] + j], w9[d, :, j] * AS)
    shared["wdiag"] = bf16(wd.transpose(1, 0, 2).reshape(128, NWD * 128))

    x = np.asarray(inputs["x"], dtype=np.float32)
    in_maps = []
    for b in range(N_CORES):
        m = dict(shared)
        m["x"] = np.ascontiguousarray(x[b])
        xp = np.zeros((128, XPW), np.float32)
        xp[:, PAD:PAD + S] = x[b].T
        m["xp0"] = bf16(xp)
        in_maps.append(m)
    return in_maps


def kernel(**inputs):
    global _NC_CACHE, LAST_RESULT
    _maybe_install_trace_shim()
    if _NC_CACHE is None:
        _NC_CACHE = build_nc()
    in_maps = _prep_inputs(inputs)
    res = run_bass_kernel_spmd(_NC_CACHE, in_maps, core_ids=list(range(N_CORES)))
    LAST_RESULT = res
    return np.stack([res.results[i]["out"] for i in range(N_CORES)], axis=0)


# revision 38
# speedup vs baseline: 1.1396x; 1.1396x over previous
"""AdaptiveTemporalKernels Trainium2 kernel (v2).

Strategy: data-parallel over batch (B=8 -> 1 element/core, zero collectives).
Weights host-side pre-transposed / pre-tiled / DoubleRow-pair-packed / cast
to fp8e4 (x128 scale); activations catT/aoT/ao2T in fp8 (x128) with
256-column branch stride so DoubleRow pair APs are 16B-aligned. Big matmuls
(qkv, attn_out, proj) run fp8 DoubleRow (K=256/instr) with f32 PSUM.

v2 changes vs baseline:
 - Depthwise convs run in bf16 on VectorE (2x_1P mode) with host-prepadded
   xpad0/xpad1 (1-col-shifted twin) so every tap slice is 4B-aligned;
   k3/k5 branches run on GpSimd in parallel; catT scale-writes moved to
   ScalarE (activation Copy with per-partition aw scale).
 - Q projection chase dropped: all-DoubleRow (PE work -37us).
 - All K=1 bias-broadcast matmuls removed: q/k/ao2 biases ride the
   PSUM->SBUF cast (DVE tensor_scalar const-mult + AP-add, or ScalarE
   Identity with AP bias); V bias folded host-side into the attn-out bias
   (softmax rows sum to 1); proj bias keeps its 2 tiny ones-matmuls.
 - Attention softmax batched: scores+exp for all heads, then all colsums,
   then per-head broadcast+aoT -- removes 8x ~1.9us PE stalls and the HAM
   re-throttle they caused.
 - proj runs token-block 0 fully before block 1 so layernorm 0 overlaps
   block 1's matmuls.
"""
import os
import sys

sys.path.insert(0, "/opt/trn_rl_repo")

import numpy as np
import ml_dtypes

import concourse.bass as bass
import concourse.tile as tile
from concourse import mybir
from concourse.bass_utils import run_bass_kernel_spmd

BF16 = mybir.dt.bfloat16
F32 = mybir.dt.float32
FP8 = mybir.dt.float8e4
DR = mybir.MatmulPerfMode.DoubleRow
AFT = mybir.ActivationFunctionType
ALU = mybir.AluOpType

KS = [3, 5, 7, 9, 11]
ND = 8
D = 128
E = 5120
H = 8
B = 8
S = 250
HD = E // H          # 640
NE = E // 128        # 40 e-tiles
CST = 256            # catT/aoT/ao2T per-branch column stride (16B-aligned pairs)
SCALE = 1.0 / float(np.sqrt(HD))
PAD = 40             # max conv halo: (11-1)*8//2
XPW = 332            # xpad width (S + 2*PAD rounded up even)
N_CORES = 8
WS = 128.0           # fp8 weight scale
AS = 128.0           # fp8 activation scale
UNS = 1.0 / (WS * AS)  # 2^-14 psum unscale

# Branch order along the catT e-axis. The k=11 branches run on the
# TensorEngine (accumulated diagonal-matmul taps, pairs 0-3) so the Q
# projection's first pass has ready pairs immediately; the remaining
# branches run on VectorE (bf16 taps at ~425ns each), paired (k3,k9) and
# (k5,k7) so every DVE pair costs the same (10 accumulate-taps). k9 stays
# on the DVE: the board power limiter trips (chip-wide ~2.0GHz downclock)
# when the PE duty is pushed much past ~85%, so extra PE work loses more
# than the idle it removes.
_PERM = []
for _d in range(ND):
    _PERM += [4 * ND + _d]                # k=11 (PE), pairs 0-3
for _d in range(ND):
    _PERM += [0 * ND + _d, 3 * ND + _d]   # k=3 with k=9 (DVE)
for _d in range(ND):
    _PERM += [1 * ND + _d, 2 * ND + _d]   # k=5 with k=7 (DVE)
assert sorted(_PERM) == list(range(40))
NPE = 8  # leading k11 branches produced on the TensorEngine
_WDOFF = [11 * _p for _p in range(NPE + 1)]
NWD = _WDOFF[NPE]  # 88

# All small constants ride ONE packed [128, CBLK_W] f32 DMA — ~18 separate
# small transfers each pay ~1.5us of ring/completion latency otherwise.
CBLK_LAYOUT = [("cw3", 24), ("cw5", 40), ("cw7", 56), ("cw9", 72), ("cw11", 88),
               ("cb3", 8), ("cb5", 8), ("cb7", 8), ("cb9", 8), ("cb11", 8),
               ("kg1", 128), ("kgb1", 1), ("kgb2", 40),
               ("bqc", 40), ("bkc", 40), ("boc", 40),
               ("gam", 128), ("bet", 128)]
CBLK_OFF = {}
_acc = 0
for _n, _w in CBLK_LAYOUT:
    CBLK_OFF[_n] = _acc
    _acc += _w
CBLK_W = _acc

LAST_RESULT = None
_NC_CACHE = None


def _split_multi_waits(nc, max_waits=1):
    """This container's walrus only lowers ONE sync-wait per instruction.
    Split any instruction carrying N>1 waits into N-1 preceding single-wait
    NoOps on the same engine."""
    import bass_rust
    SyncInfo = bass_rust.SyncInfo
    n_split = 0
    for f in nc.m.functions:
        for bb in f.blocks:
            insts = bb.instructions
            i = 0
            while i < len(insts):
                inst = insts[i]
                si = getattr(inst, "sync_info", None)
                if si is not None and si.on_wait is not None and len(si.on_wait) > max_waits:
                    waits = list(si.on_wait)
                    keep, extra = waits[-max_waits:], waits[:-max_waits]
                    nops = []
                    for w in extra:
                        nop = mybir.InstNoOp(name=f"WSPLIT-{nc.next_id()}", ins=[], outs=[])
                        nop.engine = inst.engine
                        nop.sync_info = SyncInfo(on_wait=[w], on_update=[])
                        nops.append(nop)
                    inst.sync_info = SyncInfo(on_wait=keep, on_update=list(si.on_update))
                    insts[i:i] = nops
                    i += len(nops)
                    n_split += 1
                i += 1
    return n_split


def _maybe_install_trace_shim():
    """Register the NTFF profile hook (missing antenv.axon_hooks in this image)
    so BASS_TRACE=1 yields exec_time_ns. Only used by test.py runs."""
    if not os.environ.get("BASS_TRACE"):
        return
    import types
    import antenv
    if "antenv.axon_hooks" not in sys.modules:
        mod = types.ModuleType("antenv.axon_hooks")
        mod._hook = None
        def set_axon_ntff_profile_hook(h):
            mod._hook = h
        def get_axon_ntff_profile_hook():
            return mod._hook
        mod.set_axon_ntff_profile_hook = set_axon_ntff_profile_hook
        mod.get_axon_ntff_profile_hook = get_axon_ntff_profile_hook
        sys.modules["antenv.axon_hooks"] = mod
        antenv.axon_hooks = mod
    from antenv.axon_hooks import set_axon_ntff_profile_hook
    from trn_agent_boot.trn_boot import _ntff_profile_via_ctypes
    set_axon_ntff_profile_hook(_ntff_profile_via_ctypes("/opt/axon/libaxon_pjrt.so"))
    from concourse import bass_utils
    bass_utils.upload_artifacts = lambda tmpdir: f"file://{tmpdir}"


def build_nc():
    nc = bass.Bass()

    x_ext = nc.declare_dram_parameter("x", [S, D], F32, False)
    xp0_ext = nc.declare_dram_parameter("xp0", [D, XPW], BF16, False)
    wq_ext = nc.declare_dram_parameter("wq", [10, 10, 128, 2048], FP8, False)
    wk_ext = nc.declare_dram_parameter("wk", [10, 10, 128, 2048], FP8, False)
    wv_ext = nc.declare_dram_parameter("wv", [10, 10, 128, 2048], FP8, False)
    wo_ext = nc.declare_dram_parameter("wo", [10, 10, 128, 2048], FP8, False)
    wp_ext = nc.declare_dram_parameter("wp", [128, 5120], FP8, False)
    kg2_ext = nc.declare_dram_parameter("kg2", [128, E], BF16, False)
    wd_ext = nc.declare_dram_parameter("wdiag", [128, NWD * 128], BF16, False)
    cblk_ext = nc.declare_dram_parameter("cblk", [128, CBLK_W], F32, False)
    bp_ext = nc.declare_dram_parameter("bp", [1, 128], BF16, False)
    out_ext = nc.declare_dram_parameter("out", [S, D], F32, True)

    TBLK = [(0, 128, 0), (1, 122, 128)]  # (idx, tok_len, tok_offset)

    with tile.TileContext(nc) as tc:
        with (
            tc.tile_pool(name="const", bufs=1) as cpool,
            tc.tile_pool(name="big", bufs=1) as bpool,
            tc.tile_pool(name="w", bufs=16) as wpool,
            tc.tile_pool(name="y", bufs=8) as ypool,
            tc.tile_pool(name="exp", bufs=8) as epool,
            tc.tile_pool(name="rec", bufs=8) as rpool,
            tc.tile_pool(name="ln", bufs=2) as lpool,
            tc.tile_pool(name="ps", bufs=8, space="PSUM") as pspool,
        ):
            mm = nc.tensor.matmul

            def pairv(ap_base, f, c0, c1):
                """[128, 2, c1-c0] DoubleRow view of adjacent CST-strided
                feature blocks (f, f+1)."""
                v = ap_base[:, f * CST:(f + 2) * CST]
                v = v.rearrange("p (j n) -> p j n", j=2)
                return v[:, :, c0:c1]

            # ---- constants / small inputs -------------------------------
            ones_f = cpool.tile([128, 128], F32, tag="ones_f")
            nc.vector.memset(ones_f[:], 1.0)
            scr = cpool.tile([1, 4], F32, tag="scr")
            # pre-warm the gelu act-table set while input DMAs land
            nc.scalar.activation(scr[0:1, 0:1], ones_f[0:1, 0:1], AFT.Gelu)
            ones_h = cpool.tile([128, 128], BF16, tag="ones_h")
            nc.vector.memset(ones_h[:], 1.0)
            ones_8 = cpool.tile([128, 1], FP8, tag="ones_8")
            nc.vector.memset(ones_8[:], 1.0)
            eps_sb = cpool.tile([128, 1], F32, tag="eps")
            nc.vector.memset(eps_sb[:], 1e-5)

            # DMA order is tuned to the consumption schedule: x/xp/cblk
            # first (convs start ~2us in), wdiag on the scalar ring in
            # parallel, kg2 next (the aw chain); wp is deferred until after
            # the attention section is emitted.
            x_tok = cpool.tile([128, 256], F32, tag="x_tok")
            nc.sync.dma_start(x_tok[0:128, 0:128], x_ext[0:128, :])
            nc.sync.dma_start(x_tok[0:122, 128:256], x_ext[128:250, :])
            xp0 = cpool.tile([128, XPW], BF16, tag="xp0")
            nc.sync.dma_start(xp0[:], xp0_ext[:])
            cblk = cpool.tile([128, CBLK_W], F32, tag="cblk")
            nc.sync.dma_start(cblk[:], cblk_ext[:])

            def cbk(name):
                return cblk[:, CBLK_OFF[name]:CBLK_OFF[name] + dict(CBLK_LAYOUT)[name]]

            cw_sb = {k: cbk(f"cw{k}") for k in KS}
            cb_sb = {k: cbk(f"cb{k}") for k in KS}
            kg1_sb = cbk("kg1")
            kgb1_sb = cbk("kgb1")
            kgb2_sb = cbk("kgb2")
            bqc_sb = cbk("bqc")
            bkc_sb = cbk("bkc")
            boc_sb = cbk("boc")
            gam_sb = cbk("gam")
            bet_sb = cbk("bet")
            wd_sb = cpool.tile([128, NWD * 128], BF16, tag="wd")
            for gp in range(0, NPE, 2):
                o0, o1 = _WDOFF[gp] * 128, _WDOFF[gp + 2] * 128
                nc.scalar.dma_start(wd_sb[:, o0:o1], wd_ext[:, o0:o1])
            kg2_sb = cpool.tile([128, E], BF16, tag="kg2")
            nc.sync.dma_start(kg2_sb[:], kg2_ext[:])
            bp_sb = cpool.tile([1, 128], BF16, tag="bp")
            nc.sync.dma_start(bp_sb[:], bp_ext[:])

            # ---- big persistent SBUF tensors ----------------------------
            catT = bpool.tile([128, NE * CST], FP8, tag="catT")    # [e, tok] x128
            qT = bpool.tile([128, NE * CST], FP8, tag="qT")    # (q+bq)*SCALE*AS
            kT = bpool.tile([128, NE * CST], FP8, tag="kT")    # (k+bk)*AS
            v_sb = bpool.tile([128, 2 * E], FP8, tag="v")      # [tok, blk*E+f] x128
            aoT = bpool.tile([128, NE * CST], FP8, tag="aoT")      # x128
            wp_all = bpool.tile([128, 5120], FP8, tag="wp_all")
            ao2T = bpool.tile([128, NE * CST], FP8, tag="ao2T")    # x128

            # ---- kernel generator (pt 1): h = gelu(W1 mean(x)) ----------
            ps_g = pspool.tile([128, 1], F32, tag="mm")
            mm(ps_g[:], x_tok[0:128, 0:128], ones_f[0:128, 0:1], start=True, stop=False)
            mm(ps_g[:], x_tok[0:122, 128:256], ones_f[0:122, 0:1], start=False, stop=True)
            gT = lpool.tile([128, 1], F32, tag="gT")
            nc.scalar.activation(gT[:], ps_g[:], AFT.Copy, scale=1.0 / S)
            ps_h = pspool.tile([128, 1], F32, tag="mm")
            mm(ps_h[:], kg1_sb[:], gT[:], start=True, stop=True)
            hT = lpool.tile([128, 1], BF16, tag="hT")
            nc.scalar.activation(hT[:], ps_h[:], AFT.Gelu, bias=kgb1_sb[:, 0:1])
            awT = cpool.tile([128, NE], F32, tag="awT")
            awpre = lpool.tile([128, NE], F32, tag="awpre")
            cbaw = lpool.tile([128, NPE], F32, tag="cbaw")

            # ---- depthwise convs -> catT (feature-major, fp8 x128) ------
            # k11/k9 branches (catT pairs 0-7) run on the TensorEngine as
            # accumulated diagonal matmuls; the rest run on VectorE in bf16,
            # the init tap alternating DVE/ScalarE and the catT scale-write
            # on ScalarE. Emission order is hand-scheduled so no engine FIFO
            # ever waits on the aw chain.
            ps_y = {}

            def conv_pe(p):
                k = 11
                dil = p + 1
                b0 = PAD - (k - 1) * dil // 2
                t = pspool.tile([128, S], F32, tag="mm", name=f"ps_y{p}")
                for j in range(k):
                    a = _WDOFF[p] + j
                    mm(t[:], wd_sb[:, a * 128:(a + 1) * 128],
                       xp0[:, b0 + j * dil:b0 + j * dil + S],
                       start=(j == 0), stop=(j == k - 1))
                ps_y[p] = t

            def conv_pe_write(p):
                nc.scalar.activation(catT[:, p * CST:p * CST + S], ps_y[p][:],
                                     AFT.Identity, scale=awT[:, p:p + 1],
                                     bias=cbaw[:, p:p + 1])

            def conv_dve(pos):
                ki, di = _PERM[pos] // ND, _PERM[pos] % ND
                k = KS[ki]
                dil = di + 1
                b0 = PAD - (k - 1) * dil // 2
                y = ypool.tile([128, S], BF16, tag="y")
                if pos % 2 == 0:
                    nc.scalar.activation(y[:], xp0[:, b0:b0 + S], AFT.Identity,
                                         scale=cw_sb[k][:, di * k:di * k + 1],
                                         bias=cb_sb[k][:, di:di + 1])
                else:
                    nc.vector.tensor_scalar(y[:], xp0[:, b0:b0 + S],
                                            cw_sb[k][:, di * k:di * k + 1],
                                            cb_sb[k][:, di:di + 1],
                                            ALU.mult, ALU.add)
                for j in range(1, k):
                    bj = b0 + j * dil
                    nc.vector.scalar_tensor_tensor(y[:], xp0[:, bj:bj + S],
                                                   cw_sb[k][:, di * k + j:di * k + j + 1],
                                                   y[:], ALU.mult, ALU.add)
                return y

            def conv_dve_write(pos, y):
                nc.scalar.activation(catT[:, pos * CST:pos * CST + S], y[:],
                                     AFT.Copy, scale=awT[:, pos:pos + 1])

            conv_pe(0)
            conv_pe(1)
            dpend = [(pos, conv_dve(pos)) for pos in range(NPE, NPE + 4)]
            # kernel generator (pt 2): aw = tanh(W2 h + b2) — the 40 matmuls
            # land between PE conv branches so the PE never idles on gelu
            ps_aw = pspool.tile([128, NE], F32, tag="mm")
            for blk in range(NE):
                mm(ps_aw[:, blk:blk + 1], kg2_sb[:, blk * 128:(blk + 1) * 128], hT[:],
                   start=True, stop=True)
            nc.vector.tensor_add(awpre[:], ps_aw[:], kgb2_sb[:])
            nc.scalar.activation(awT[:], awpre[:], AFT.Tanh)
            # cbaw[:, p] = awT[:, p] * cb[branch p] for the PE branches
            nc.vector.tensor_mul(cbaw[:], awT[:, 0:NPE], cb_sb[11][:, 0:ND])
            conv_pe(2)
            conv_pe(3)
            for p in range(4):
                conv_pe_write(p)
            for p in range(4, NPE):
                conv_pe(p)
                conv_pe_write(p)
            for pos, y in dpend:
                conv_dve_write(pos, y)
            # short HAM warmup bridge across the catT-write drain
            ps_w = pspool.tile([128, 128], F32, tag="mm", name="warm")
            for i in range(4):
                mm(ps_w[:], ones_f[:], ones_f[:], start=True, stop=True)
            for pos in range(NPE + 4, NE):
                y = conv_dve(pos)
                conv_dve_write(pos, y)
            # pre-warm the exp act-table set (used next in attention) while
            # the projections run; Copy/Identity are fillers in every set
            nc.scalar.activation(scr[0:1, 1:2], ones_f[0:1, 0:1], AFT.Exp)

            # ---- qT / kT / ao2T: feature-major fp8 DoubleRow ------------
            def qk_like(wext, bias_col, dest, cast_s, src_act, NG=4):
                # NG g-groups interleaved in the PE stream so the conv-paced
                # prologue always has ready matmul work; 2 fblks share one
                # PSUM bank at 256-column halves ([128,512] f32 = 1 bank);
                # bias + unscale ride the PSUM->SBUF cast, alternating
                # DVE (tensor_scalar mult-const + add-AP) and ScalarE
                # (Identity with AP bias) per feature block.
                for g0 in range(0, 10, NG):
                    gs = list(range(g0, min(10, g0 + NG)))
                    ps = {g: [pspool.tile([128, 512], F32, tag="mm",
                                          name=f"ps_{dest.tensor.name}_{g}_{i}")
                              for i in range(2)] for g in gs}
                    for e4 in range(10):
                        wts = {}
                        for gi, g in enumerate(gs):
                            wt = wpool.tile([128, 2048], FP8, tag="w")
                            eng = nc.sync if gi % 2 == 0 else nc.scalar
                            eng.dma_start(wt[:], wext[g, e4])
                            wts[g] = wt
                        for ep in range(2):
                            pr = e4 * 2 + ep
                            rhs = pairv(src_act, 2 * pr, 0, S)
                            for g in gs:
                                for j in range(4):
                                    lh = wts[g][:, ep * 1024 + j * 256:ep * 1024 + (j + 1) * 256]
                                    lh = lh.rearrange("p (j n) -> p j n", j=2)
                                    mm(ps[g][j // 2][:, (j % 2) * 256:(j % 2) * 256 + S],
                                       lh, rhs,
                                       start=(pr == 0), stop=(pr == 19), perf_mode=DR,
                                       skip_group_check=True)
                    for g in gs:
                        for i in range(2):
                            for h2 in range(2):
                                fb = g * 4 + i * 2 + h2
                                sl_d = dest[:, fb * CST:fb * CST + S]
                                sl_p = ps[g][i][:, h2 * 256:h2 * 256 + S]
                                if fb % 2 == 0:
                                    nc.vector.tensor_scalar(
                                        sl_d, sl_p, cast_s, bias_col[:, fb:fb + 1],
                                        ALU.mult, ALU.add)
                                else:
                                    nc.scalar.activation(
                                        sl_d, sl_p, AFT.Identity,
                                        bias=bias_col[:, fb:fb + 1], scale=cast_s)

            qk_like(wq_ext, bqc_sb, qT, UNS * SCALE * AS, catT)
            qk_like(wk_ext, bkc_sb, kT, UNS * AS, catT)

            # ---- V: token-major fp8 DoubleRow (lhsT = catT pairs) -------
            # v stored fp8 at x128 scale (psum * 2^-7); bias folded into the
            # attn-out bias host-side (softmax rows sum to 1).
            # dead rows of the short token block must be finite zeros: the
            # attention DR pair contraction multiplies them by exp's zeroed
            # pad rows, and 0 * garbage-inf would be NaN
            nc.gpsimd.memset(v_sb[96:128, E:2 * E], 0.0)
            for g in range(10):
                psv = [pspool.tile([128, 512], F32, tag="mm", name=f"psv_{g}_{i}") for i in range(2)]
                for e4 in range(10):
                    wt = wpool.tile([128, 2048], FP8, tag="w")
                    eng = nc.sync if e4 % 2 == 0 else nc.scalar
                    eng.dma_start(wt[:], wv_ext[g, e4])
                    for ep in range(2):
                        pr = e4 * 2 + ep
                        rh = wt[:, ep * 1024:(ep + 1) * 1024]
                        rh = rh.rearrange("p (j n) -> p j n", j=2)
                        mm(psv[0][:], pairv(catT, 2 * pr, 0, 128), rh,
                           start=(pr == 0), stop=(pr == 19), perf_mode=DR)
                        mm(psv[1][0:122, :], pairv(catT, 2 * pr, 128, 250), rh,
                           start=(pr == 0), stop=(pr == 19), perf_mode=DR)
                nc.vector.tensor_scalar_mul(
                    v_sb[0:128, g * 512:(g + 1) * 512], psv[0][:], UNS * AS)
                nc.scalar.activation(
                    v_sb[0:122, E + g * 512:E + (g + 1) * 512], psv[1][0:122, :],
                    AFT.Copy, scale=UNS * AS)
            v_pair = v_sb.rearrange("p (j f) -> p j f", j=2)

            # ---- attention (bf16-ish; batched two-pass softmax) ---------
            # Pass A: scoresT + exp for every head; Pass B: all colsums,
            # then per-head reciprocal-broadcast + aoT.
            # Lag-pipelined attention: per iteration i the PE runs scores(i),
            # colsum(i-2), reciprocal-broadcast(i-3) and aoT(i-4), so the
            # softmax chain of each head hides behind other heads' matmuls
            # and the PE busy-density stays high (no HAM re-throttle).
            exs, exns, recips = [], [], []
            for h in range(H):
                ex = epool.tile([128, 512], FP8, tag="exp", name=f"ex{h}")
                # zero the short token-block's dead rows so the DR pair
                # contraction reads 0 * garbage there
                nc.gpsimd.memset(ex[96:128, 256:512], 0.0)
                exs.append(ex)
                exn = epool.tile([128, 512], FP8, tag="exn", name=f"exn{h}")
                nc.gpsimd.memset(exn[96:128, 256:512], 0.0)
                exns.append(exn)

            def att_scores(h):
                for kb, klen, koff in TBLK:
                    ps_s = pspool.tile([128, S], F32, tag="mm", name=f"ps_s{h}_{kb}")
                    for dp in range(2):
                        f = h * 5 + dp * 2
                        mm(ps_s[0:klen, :],
                           pairv(kT, f, koff, koff + klen),
                           pairv(qT, f, 0, S),
                           start=(dp == 0), stop=False, perf_mode=DR,
                           skip_group_check=True)
                    f4 = (h * 5 + 4) * CST
                    mm(ps_s[0:klen, :],
                       kT[:, f4 + koff:f4 + koff + klen],
                       qT[:, f4:f4 + S],
                       start=False, stop=True, skip_group_check=True)
                    nc.scalar.activation(exs[h][0:klen, kb * 256:kb * 256 + S],
                                         ps_s[0:klen, :], AFT.Exp, scale=UNS)

            def att_colsum(h):
                ps_sum = pspool.tile([1, S], F32, tag="mm", name=f"ps_sum{h}")
                mm(ps_sum[:], ones_8[0:128, 0:1], exs[h][0:128, 0:S],
                   start=True, stop=False, skip_group_check=True)
                mm(ps_sum[:], ones_8[0:122, 0:1], exs[h][0:122, 256:256 + S],
                   start=False, stop=True, skip_group_check=True)
                recip = rpool.tile([1, S], F32, tag="recip", name=f"recip{h}")
                nc.vector.reciprocal(recip[:], ps_sum[:])
                recips.append(recip)

            def att_bcast_norm(h):
                ps_b = pspool.tile([128, S], F32, tag="mm", name=f"ps_b{h}")
                mm(ps_b[:], ones_f[0:1, 0:128], recips[h][0:1, :], start=True, stop=True)
                nc.vector.tensor_mul(exns[h][0:128, 0:S], exs[h][0:128, 0:S],
                                     ps_b[0:128, :])
                nc.vector.tensor_mul(exns[h][0:122, 256:256 + S],
                                     exs[h][0:122, 256:256 + S], ps_b[0:122, :])

            def att_ao(h):
                # two dblks share one [128,512] PSUM bank at 256-col halves
                # so each cast covers both; casts alternate ScalarE/DVE so
                # neither engine paces the pipeline below PE speed
                ex_pair = exns[h].rearrange("p (j n) -> p j n", j=2)
                for dd in range(3):
                    nd = 2 if dd < 2 else 1
                    ps_ao = pspool.tile([128, 512], F32, tag="mm", name=f"ps_ao{h}_{dd}")
                    for q in range(nd):
                        dblk = dd * 2 + q
                        c0 = h * HD + dblk * 128
                        mm(ps_ao[:, q * 256:q * 256 + S],
                           v_pair[:, :, c0:c0 + 128], ex_pair[:, :, 0:S],
                           start=True, stop=True, perf_mode=DR, skip_group_check=True)
                    e = h * 5 + dd * 2
                    dst = aoT[:, e * CST:(e + nd) * CST].rearrange("p (j n) -> p j n", j=nd)
                    srcv = ps_ao[:].rearrange("p (j n) -> p j n", j=2)[:, 0:nd, 0:S]
                    if (dd + h) % 2 == 0:
                        nc.scalar.activation(dst[:, :, 0:S], srcv, AFT.Copy, scale=1.0)
                    else:
                        nc.vector.tensor_scalar_mul(dst[:, :, 0:S], srcv, 1.0)

            ps_af = pspool.tile([128, 128], F32, tag="mm", name="attn_fill")
            for i in range(H + 4):
                if i < H:
                    att_scores(i)
                if 0 <= i - 2 < H:
                    att_colsum(i - 2)
                if 0 <= i - 3 < H:
                    att_bcast_norm(i - 3)
                if 0 <= i - 4 < H:
                    att_ao(i - 4)
                mm(ps_af[:], ones_f[:], ones_f[:], start=True, stop=True)
                if i == H - 1:
                    # pre-warm the sqrt act-table set (layernorm) off the
                    # critical tail; Copy/Identity/Square are fillers there
                    nc.scalar.activation(scr[0:1, 2:3], ones_f[0:1, 0:1], AFT.Sqrt)

            # deferred wp DMA (needed only by proj) — lands during the
            # low-DMA attention phase, ahead of ao2's stream
            nc.scalar.dma_start(wp_all[:], wp_ext[:])

            # ---- ao2T: feature-major fp8 DoubleRow (lhsT = Wo pairs) ----
            qk_like(wo_ext, boc_sb, ao2T, UNS * AS, aoT)

            # ---- final proj fp8 DoubleRow + residual + layernorm --------
            # token-block 0's full accumulation chain runs first so its
            # layernorm overlaps block 1's matmuls.
            psf = [pspool.tile([128, 128], F32, tag="mm", name=f"psf_{i}") for i in range(2)]
            ao2_pair = ao2T  # alias for clarity
            for tb, tlen, toff in TBLK:
                for pr in range(20):
                    rh = wp_all[:, pr * 256:(pr + 1) * 256]
                    rh = rh.rearrange("p (j n) -> p j n", j=2)
                    mm(psf[tb][0:tlen, :], pairv(ao2_pair, 2 * pr, toff, toff + tlen), rh,
                       start=(pr == 0), stop=False, perf_mode=DR)
                mm(psf[tb][0:tlen, :], ones_h[0:1, 0:tlen], bp_sb[0:1, :],
                   start=False, stop=True)

                ln_in = lpool.tile([128, 128], F32, tag="ln_in")
                redsum = lpool.tile([128, 1], F32, tag="redsum")
                nc.vector.scalar_tensor_tensor(
                    ln_in[0:tlen, :], psf[tb][0:tlen, :], UNS / AS,
                    x_tok[0:tlen, toff:toff + 128], ALU.mult, ALU.add,
                    accum_out=redsum[0:tlen, :])
                negmean = lpool.tile([128, 1], F32, tag="negmean")
                nc.scalar.activation(negmean[0:tlen, :], redsum[0:tlen, :],
                                     AFT.Copy, scale=-1.0 / D)
                cent = lpool.tile([128, 128], F32, tag="cent")
                nc.vector.tensor_scalar_add(cent[0:tlen, :], ln_in[0:tlen, :],
                                            negmean[0:tlen, 0:1])
                sq = lpool.tile([128, 128], F32, tag="sq")
                varsum = lpool.tile([128, 1], F32, tag="varsum")
                nc.scalar.activation(sq[0:tlen, :], cent[0:tlen, :], AFT.Square,
                                     accum_out=varsum[0:tlen, :])
                std = lpool.tile([128, 1], F32, tag="std")
                nc.scalar.activation(std[0:tlen, :], varsum[0:tlen, :], AFT.Sqrt,
                                     scale=1.0 / D, bias=eps_sb[0:tlen, 0:1])
                rstd = lpool.tile([128, 1], F32, tag="rstd")
                nc.vector.reciprocal(rstd[0:tlen, :], std[0:tlen, :])
                gmm = lpool.tile([128, 128], F32, tag="gmm")
                nc.vector.scalar_tensor_tensor(
                    gmm[0:tlen, :], cent[0:tlen, :], rstd[0:tlen, 0:1],
                    gam_sb[0:tlen, :], ALU.mult, ALU.mult)
                outf = lpool.tile([128, 128], F32, tag="outf")
                nc.vector.tensor_add(outf[0:tlen, :], gmm[0:tlen, :], bet_sb[0:tlen, :])
                nc.sync.dma_start(out_ext[toff:toff + tlen, :], outf[0:tlen, :])

    _split_multi_waits(nc)
    return nc


def _prep_inputs(inputs):
    f32 = lambda a: np.ascontiguousarray(np.asarray(a, dtype=np.float32))
    bf16 = lambda a: np.ascontiguousarray(np.asarray(a, dtype=np.float32).astype(ml_dtypes.bfloat16))
    fp8 = lambda a: np.ascontiguousarray(np.asarray(a, dtype=np.float32).astype(ml_dtypes.float8_e4m3))

    def perm_k(wT):   # permute contraction-axis 128-blocks by _PERM
        n = wT.shape[1]
        return wT.reshape(NE, 128, n)[_PERM].reshape(E, n)

    def dr_lhs(wT, permute=False):  # [E(K), N] -> [g, e4, 128p, (ep,fj,j,m)] DR lhsT pairs
        if permute:
            wT = perm_k(wT)
        n = wT.shape[1]
        return (wT.reshape(10, 2, 2, 128, n // 512, 4, 128)
                .transpose(4, 0, 3, 1, 5, 2, 6).reshape(n // 512, 10, 128, 2048))

    def dr_rhs(wT, permute=False):  # [E(K), N] -> [g, e4, 128p, (ep,j,c)] DR rhs pairs
        if permute:
            wT = perm_k(wT)
        n = wT.shape[1]
        return (wT.reshape(10, 2, 2, 128, n // 512, 512)
                .transpose(4, 0, 3, 1, 2, 5).reshape(n // 512, 10, 128, 2048))

    def col128(vec, scale):  # [E] bias -> [128, NE] per-partition columns
        return f32(np.asarray(vec, np.float32).reshape(NE, 128).T * scale)

    A = np.asarray(inputs["attn_in_w"], dtype=np.float32)
    Wo = np.asarray(inputs["attn_out_w"], np.float32)
    bv = np.asarray(inputs["attn_in_b"], np.float32)[2 * E:3 * E]
    bo_eff = np.asarray(inputs["attn_out_b"], np.float32) + Wo @ bv
    shared = {
        "wq": fp8(dr_lhs(A[0:E].T, permute=True) * WS),
        "wk": fp8(dr_lhs(A[E:2 * E].T, permute=True) * WS),
        "wv": fp8(dr_rhs(A[2 * E:3 * E].T, permute=True) * WS),
        "wo": fp8(dr_lhs(Wo.T) * WS),
        "wp": fp8(np.asarray(inputs["proj_w"], np.float32).T
                  .reshape(10, 2, 2, 128, 128).transpose(3, 0, 1, 2, 4)
                  .reshape(128, 5120) * WS),
        "kg2": bf16(np.asarray(inputs["kg_w2"], np.float32).T.reshape(128, NE, 128)[:, _PERM, :].reshape(128, E)),
        "bp": bf16(np.asarray(inputs["proj_b"]).reshape(1, 128) * WS * AS * AS),
    }
    # packed small-constant block (single DMA); conv taps/bias pre-scaled by
    # AS so y is produced at x128 scale and the catT write is a pure
    # aw-scale copy
    cblk = np.zeros((128, CBLK_W), np.float32)

    def put(name, arr):
        o = CBLK_OFF[name]
        cblk[:, o:o + arr.shape[1]] = arr

    for k in KS:
        put(f"cw{k}", np.asarray(inputs[f"conv_w_k{k}"], np.float32)
            .transpose(1, 0, 2).reshape(128, ND * k) * AS)
        put(f"cb{k}", np.asarray(inputs[f"conv_b_k{k}"], np.float32).T * AS)
    put("kg1", np.asarray(inputs["kg_w1"], np.float32).T)
    put("kgb1", np.asarray(inputs["kg_b1"], np.float32).reshape(128, 1))
    put("kgb2", np.asarray(inputs["kg_b2"], np.float32).reshape(NE, 128)[_PERM].T)
    put("bqc", col128(np.asarray(inputs["attn_in_b"], np.float32)[0:E], SCALE * AS))
    put("bkc", col128(np.asarray(inputs["attn_in_b"], np.float32)[E:2 * E], AS))
    put("boc", col128(bo_eff, AS))
    put("gam", np.broadcast_to(np.asarray(inputs["gamma"], np.float32), (128, 128)))
    put("bet", np.broadcast_to(np.asarray(inputs["beta"], np.float32), (128, 128)))
    shared["cblk"] = f32(cblk)
    # diagonal tap matrices for the PE conv branches (k11, dil order)
    w11 = np.asarray(inputs["conv_w_k11"], np.float32)  # [ND, 128, 11]
    wd = np.zeros((NWD, 128, 128), np.float32)
    for d in range(ND):
        for j in range(11):
            np.fill_diagonal(wd[_WDOFF[d] + j], w11[d, :, j] * AS)
    shared["wdiag"] = bf16(wd.transpose(1, 0, 2).reshape(128, NWD * 128))

    x = np.asarray(inputs["x"], dtype=np.float32)
    in_maps = []
    for b in range(N_CORES):
        m = dict(shared)
        m["x"] = np.ascontiguousarray(x[b])
        xp = np.zeros((128, XPW), np.float32)
        xp[:, PAD:PAD + S] = x[b].T
        m["xp0"] = bf16(xp)
        in_maps.append(m)
    return in_maps


def kernel(**inputs):
    global _NC_CACHE, LAST_RESULT
    _maybe_install_trace_shim()
    if _NC_CACHE is None:
        _NC_CACHE = build_nc()
    in_maps = _prep_inputs(inputs)
    res = run_bass_kernel_spmd(_NC_CACHE, in_maps, core_ids=list(range(N_CORES)))
    LAST_RESULT = res
    return np.stack([res.results[i]["out"] for i in range(N_CORES)], axis=0)
